# revision 12
# baseline (speedup 1.0000x reference)
"""GRU decoder kernel for Trainium2 (8 NeuronCores, data-parallel over batch).

Math (PyTorch GRU, gate order r,z,n), per batch element:
    gx_t = x_t * w_ih + b_ih              (input dim == 1 -> rank-1)
    gh_t = h_{t-1} @ w_hh.T + b_hh
    r = sigmoid(gx_r + gh_r); z = sigmoid(gx_z + gh_z)
    n = tanh(gx_n + b_ih_n + r * (gh_n + b_hh_n))
    h_t = (1-z)*n + z*h_{t-1}
    out = h_T @ fc_w.T + fc_b

v2 layout (per core, B_c = 1024 batch):
  - partition-stacked: batch 0-511 ("u") on SBUF partitions 0-63,
    batch 512-1023 ("v") on partitions 64-127 for H and all gate tensors.
  - h-matmuls: block-diag lhsT diag(Wg.T, Wg.T) [128,128], K=128 covering
    both halves in one pass per gate.
  - x + bias contribution: X4 tile interleaves (u-x, v-x, ones, pad) with
    period 4 on partitions; per step a K=32 matmul with a mostly-zero
    [32,128] weight slice adds w_g*x + b_g for both halves (biases ride
    the ones-row -> bias-free merged sigmoid).
  - PSUM bank packing: bankRZ [128,512] = r|z pre-acts -> ONE merged
    sigmoid; bankNX [128,512] = hn|xn.
  - xn drained to fp16 by Scalar engine so the T2 add runs at DVE 2x.
  - U = h - n runs on GpSimd (otherwise idle engine).
  - NGROUP phase-shifted batch groups pipeline the serial chain.
"""

import os
import sys

sys.path.insert(0, "/opt/trn_rl_repo")

import numpy as np
from contextlib import ExitStack

HIDDEN = 64
OUT = 256
B = 8192
T = int(os.environ.get("GRU_T", 1024))
NCORES = 8
BC = B // NCORES          # 1024 batch per core
HB = BC // 2              # 512 batch per partition-half
UNROLL = 32               # steps per block (4 partitions per step in X4)
NGROUP = int(os.environ.get("GRU_NGROUP", 2))  # phase-shifted batch groups
NBLK = T // UNROLL        # number of blocks

_CACHE = {}


def _build():
    import concourse.bass as bass
    import concourse.tile as tile
    from concourse import bacc, mybir

    f16 = mybir.dt.float16
    f32 = mybir.dt.float32
    AF = mybir.ActivationFunctionType
    OP = mybir.AluOpType

    nc = bacc.Bacc("TRN2", target_bir_lowering=False, debug=False,
                   num_devices=NCORES)

    d_x = nc.dram_tensor("xt", [128, NBLK, HB], f16, kind="ExternalInput").ap()
    d_dr = nc.dram_tensor("dr", [128, 128], f16, kind="ExternalInput").ap()
    d_dz = nc.dram_tensor("dz", [128, 128], f16, kind="ExternalInput").ap()
    d_dn = nc.dram_tensor("dn", [128, 128], f16, kind="ExternalInput").ap()
    d_xwr = nc.dram_tensor("xwr", [128, 8 * 128], f16, kind="ExternalInput").ap()
    d_xwz = nc.dram_tensor("xwz", [128, 8 * 128], f16, kind="ExternalInput").ap()
    d_xwn = nc.dram_tensor("xwn", [128, 8 * 128], f16, kind="ExternalInput").ap()
    d_bnh = nc.dram_tensor("bnh", [128, 1], f32, kind="ExternalInput").ap()
    d_fcw = nc.dram_tensor("fcw", [128, OUT], f16, kind="ExternalInput").ap()
    d_fcb = nc.dram_tensor("fcb", [128, 2], f32, kind="ExternalInput").ap()
    d_out = nc.dram_tensor("out", [OUT, BC], f32, kind="ExternalOutput").ap()

    with tile.TileContext(nc) as tc, ExitStack() as ctx:
        singles = ctx.enter_context(tc.tile_pool(name="singles", bufs=1))
        work = ctx.enter_context(tc.tile_pool(name="work", bufs=2))
        psum = ctx.enter_context(tc.tile_pool(name="psum", bufs=1, space="PSUM"))

        X = singles.tile([128, NBLK, HB], f16)
        DR = singles.tile([128, 128], f16)
        DZ = singles.tile([128, 128], f16)
        DN = singles.tile([128, 128], f16)
        XWR = singles.tile([128, 8 * 128], f16)
        XWZ = singles.tile([128, 8 * 128], f16)
        XWN = singles.tile([128, 8 * 128], f16)
        BNH = singles.tile([128, 1], f32)
        FCW = singles.tile([128, OUT], f16)
        FCB = singles.tile([128, 2], f32)
        HG = HB // NGROUP   # free-dim width per pipelined batch group
        # one H tile per group: groups must not share a tile or the
        # dependency tracker serializes their chains
        Hs = [singles.tile([128, HG], f16, name=f"H{g}")
              for g in range(NGROUP)]

        for dst, src in ((X, d_x), (DR, d_dr), (DZ, d_dz), (DN, d_dn),
                         (XWR, d_xwr), (XWZ, d_xwz), (XWN, d_xwn),
                         (BNH, d_bnh), (FCW, d_fcw), (FCB, d_fcb)):
            nc.gpsimd.dma_start(dst[:], src[:])
        for Hg in Hs:
            nc.vector.memset(Hg[:], 0.0)

        def mm_noldw(out, lhsT, rhs, start, stop, tile_position=None):
            """matmul that reuses the weights loaded by the previous matmul
            (InstMatmult.ldweights=False -> codegen emits no LDWEIGHTS)."""
            te = nc.tensor
            orig = te.add_instruction

            def patched(ins, **kw):
                if isinstance(ins, mybir.InstMatmult):
                    ins.ldweights = False
                return orig(ins, **kw)

            te.add_instruction = patched
            try:
                return te.matmul(out, lhsT, rhs, start=start, stop=stop,
                                 tile_position=tile_position)
            finally:
                del te.add_instruction

        rr = slice(0, HG)
        zz = slice(HG, 2 * HG)
        G = list(range(NGROUP))

        def step_mms(q, blk):
            strip = 32 * (q // 8)
            qq = q % 8
            ksl = slice(strip, strip + 32)
            wsl = slice(qq * 128, (qq + 1) * 128)
            xtp = (strip, 0)
            banksRZ = [psum.tile([128, 2 * HG], f32, tag=f"bankRZ{g}",
                                 name=f"bankRZ{g}") for g in G]
            banksNX = [psum.tile([128, 2 * HG], f32, tag=f"bankNX{g}",
                                 name=f"bankNX{g}") for g in G]
            # Gate-major pairs: both groups' matmuls for one weight set are
            # adjacent; the second reuses the loaded weights (no LDWEIGHTS).
            # Per-PSUM-tile accumulation groups stay sequential (r closes
            # before z opens); x leads h so it runs during the previous
            # step's elementwise phase.
            units = (
                (XWR[ksl, wsl], None, banksRZ, rr, True, False, xtp),
                (DR[:], Hs, banksRZ, rr, False, True, None),
                (XWN[ksl, wsl], None, banksNX, zz, True, True, xtp),
                (DN[:], Hs, banksNX, rr, True, True, None),
                (XWZ[ksl, wsl], None, banksRZ, zz, True, False, xtp),
                (DZ[:], Hs, banksRZ, zz, False, True, None),
            )
            for w, hs, banks, csl, st, sp, tp in units:
                for g in G:
                    rhs = hs[g][:] if hs is not None else X[ksl, blk,
                                                            slice(g * HG, (g + 1) * HG)]
                    if g == 0:
                        nc.tensor.matmul(banks[g][:, csl], w, rhs, start=st,
                                         stop=sp, tile_position=tp)
                    else:
                        mm_noldw(banks[g][:, csl], w, rhs, start=st, stop=sp,
                                 tile_position=tp)
            return banksRZ, banksNX

        def step_ew(banksRZ, banksNX):
            SR = [work.tile([128, HG], f16, tag=f"SR{g}", name=f"SR{g}") for g in G]
            SZ = [work.tile([128, HG], f16, tag=f"SZ{g}", name=f"SZ{g}") for g in G]
            XN = [work.tile([128, HG], f16, tag=f"XN{g}", name=f"XN{g}") for g in G]
            T1 = [work.tile([128, HG], f16, tag=f"T1{g}", name=f"T1{g}") for g in G]
            T2 = [work.tile([128, HG], f16, tag=f"T2{g}", name=f"T2{g}") for g in G]
            NN = [work.tile([128, HG], f16, tag=f"NN{g}", name=f"NN{g}") for g in G]
            U = [work.tile([128, HG], f16, tag=f"U{g}", name=f"U{g}") for g in G]
            V = [work.tile([128, HG], f16, tag=f"V{g}", name=f"V{g}") for g in G]
            for g in G:
                nc.scalar.activation(SR[g][:], banksRZ[g][:, rr], AF.Sigmoid)
            for g in G:
                nc.scalar.activation(XN[g][:], banksNX[g][:, zz], AF.Identity)
            for g in G:
                nc.scalar.activation(SZ[g][:], banksRZ[g][:, zz], AF.Sigmoid)
            for g in G:
                # T1 = (hn + b_hh_n) * r
                nc.vector.scalar_tensor_tensor(T1[g][:], banksNX[g][:, rr],
                                               BNH[:], SR[g][:],
                                               op0=OP.add, op1=OP.mult)
            for g in G:
                nc.vector.tensor_add(T2[g][:], T1[g][:], XN[g][:])
            for g in G:
                nc.scalar.activation(NN[g][:], T2[g][:], AF.Tanh)
            # h' = n + z*(h - n)
            for g in G:
                nc.vector.tensor_sub(U[g][:], Hs[g][:], NN[g][:])
            for g in G:
                nc.vector.tensor_mul(V[g][:], SZ[g][:], U[g][:])
            for g in G:
                nc.vector.tensor_add(Hs[g][:], NN[g][:], V[g][:])

        def body(blk):
            for q in range(UNROLL):
                brz, bnx = step_mms(q, blk)
                step_ew(brz, bnx)

        if NBLK == 1:
            body(0)
        else:
            with tc.For_i(0, NBLK, 1,
                          hint_engines=(mybir.EngineType.PE,)) as i:
                body(bass.ds(i, 1))

        # Final FC: out[o, b] = sum_k fc_w[o, k] h[b, k] + fc_b[o]
        for oh in range(2):
            osl = slice(oh * 128, (oh + 1) * 128)
            for g in range(NGROUP):
                H = Hs[g]
                fc_u = psum.tile([128, HG], f32, tag="bankRZ0")
                fc_v = psum.tile([128, HG], f32, tag="bankNX0")
                nc.tensor.matmul(fc_u[:], FCW[0:64, osl], H[0:64, :],
                                 start=True, stop=True, tile_position=(0, 0))
                nc.tensor.matmul(fc_v[:], FCW[64:128, osl], H[64:128, :],
                                 start=True, stop=True, tile_position=(64, 0))
                Ou = work.tile([128, HG], f32, tag="Ou")
                Ov = work.tile([128, HG], f32, tag="Ov")
                nc.scalar.activation(Ou[:], fc_u[:], AF.Identity,
                                     bias=FCB[:, oh:oh + 1])
                nc.scalar.activation(Ov[:], fc_v[:], AF.Identity,
                                     bias=FCB[:, oh:oh + 1])
                gd = slice(g * HG, (g + 1) * HG)
                gdv = slice(HB + g * HG, HB + (g + 1) * HG)
                nc.gpsimd.dma_start(d_out[osl, gd], Ou[:])
                nc.gpsimd.dma_start(d_out[osl, gdv], Ov[:])

    nc.compile()
    return nc


def _host_inputs(x, w_ih, w_hh, b_ih, b_hh, fc_w, fc_b):
    """Build the per-core in_maps (numpy, laid out exactly as SBUF tiles)."""
    f16 = np.float16
    f32 = np.float32
    x = np.asarray(x, f32)
    w_ih = np.asarray(w_ih, f32)
    w_hh = np.asarray(w_hh, f32)
    b_ih = np.asarray(b_ih, f32)
    b_hh = np.asarray(b_hh, f32)
    fc_w = np.asarray(fc_w, f32)
    fc_b = np.asarray(fc_b, f32)

    def diag2(seg):
        t = w_hh[seg, :].T                      # [64(k), 64(m)]
        d = np.zeros((128, 128), f32)
        d[0:64, 0:64] = t
        d[64:128, 64:128] = t
        return d.astype(f16)

    def xw(seg, bias):
        # [32, 8, 128]: row 4*qq+r within a strip, step-in-strip qq
        w = w_ih[seg, 0]                        # [64]
        b = bias                                # [64]
        m = np.zeros((32, 8, 128), f32)
        for qq in range(8):
            m[4 * qq + 0, qq, 0:64] = w
            m[4 * qq + 1, qq, 64:128] = w
            m[4 * qq + 2, qq, 0:64] = b
            m[4 * qq + 2, qq, 64:128] = b
        m = m.reshape(32, 8 * 128)
        return np.tile(m, (4, 1)).astype(f16)   # [128, 1024] (4 strips)

    shared = {
        "dr": diag2(slice(0, 64)),
        "dz": diag2(slice(64, 128)),
        "dn": diag2(slice(128, 192)),
        "xwr": xw(slice(0, 64), b_ih[0:64] + b_hh[0:64]),
        "xwz": xw(slice(64, 128), b_ih[64:128] + b_hh[64:128]),
        "xwn": xw(slice(128, 192), b_ih[128:192]),
        "bnh": np.tile(b_hh[128:192].reshape(-1, 1), (2, 1)).astype(f32),
        "fcw": np.vstack([fc_w.T, fc_w.T]).astype(f16),  # [128, 256]
        "fcb": np.stack([fc_b[0:128], fc_b[128:256]], 1).astype(f32),
    }

    in_maps = []
    for c in range(NCORES):
        xs = x[c * BC:(c + 1) * BC, :T, 0]            # [BC b, T t]
        xT = np.ascontiguousarray(xs.T)               # [T, BC]
        xr = xT.reshape(NBLK, UNROLL, BC)             # [blk, q, b]
        X4 = np.zeros((128, NBLK, HB), f32)
        qs = np.arange(UNROLL)
        X4[4 * qs + 0, :, :] = xr[:, :, 0:HB].transpose(1, 0, 2)
        X4[4 * qs + 1, :, :] = xr[:, :, HB:BC].transpose(1, 0, 2)
        X4[4 * qs + 2, :, :] = 1.0
        m = dict(shared)
        m["xt"] = X4.astype(f16)
        in_maps.append(m)
    return in_maps


def _run(in_maps, trace=False):
    from concourse import bass_utils
    if "nc" not in _CACHE:
        _CACHE["nc"] = _build()
    nc = _CACHE["nc"]
    res = bass_utils.run_bass_kernel_spmd(
        nc, in_maps, core_ids=list(range(NCORES)), trace=trace)
    return res


def kernel(**inputs):
    in_maps = _host_inputs(**inputs)
    res = _run(in_maps, trace=False)
    out = np.empty([B, OUT], np.float32)
    for c in range(NCORES):
        out[c * BC:(c + 1) * BC, :] = res.results[c]["out"].T
    return out


# revision 13
# speedup vs baseline: 1.4725x; 1.4725x over previous
"""GRU decoder kernel for Trainium2 (8 NeuronCores, data-parallel over batch).

Math (PyTorch GRU, gate order r,z,n), per batch element:
    gx_t = x_t * w_ih + b_ih              (input dim == 1 -> rank-1)
    gh_t = h_{t-1} @ w_hh.T + b_hh
    r = sigmoid(gx_r + gh_r); z = sigmoid(gx_z + gh_z)
    n = tanh(gx_n + b_ih_n + r * (gh_n + b_hh_n))
    h_t = (1-z)*n + z*h_{t-1}
    out = h_T @ fc_w.T + fc_b

Device layout (per core, B_c = 1024 batch):
  - partition-stacked: batch 0-511 ("u") on SBUF partitions 0-63,
    batch 512-1023 ("v") on partitions 64-127.  All elementwise ops are
    [128, HG] (gate dim on partitions within each half, batch on free dim).
  - Two PE quadrant chains: u at tile (0,0), v at (64,64); M=64 matmuls
    (~154 ns each incl weight load).
  - x contribution per step via a one-hot K=64 matmul (selects timestep q
    from the 64-step block streamed on partitions).
  - Critical-chain schedule: per step the r-region matmuls come first
    (x-part leads each PSUM accumulation pair so it runs during the
    previous step's elementwise phase), then n, then z; the h-update is
    the 2-op form h' = (1-z)*n + z*h with z*h computed on the otherwise
    idle GpSimd engine off the critical chain.
  - fp16 SBUF tensors, fp32 PSUM accumulation; NGROUP phase-shifted
    batch groups pipeline the serial chain.
"""

import os
import sys

sys.path.insert(0, "/opt/trn_rl_repo")

import numpy as np
from contextlib import ExitStack

HIDDEN = 64
OUT = 256
B = 8192
T = int(os.environ.get("GRU_T", 1024))
NCORES = 8
BC = B // NCORES          # 1024 batch per core
HB = BC // 2              # 512 batch per partition-half
UNROLL = 64               # steps per loop body (one-hot q index is static)
NGROUP = int(os.environ.get("GRU_NGROUP", 2))  # phase-shifted batch groups
M2ENG = os.environ.get("GRU_M2ENG", "gpsimd")  # engine for z*h
NBLK = T // UNROLL        # number of 64-step blocks

_CACHE = {}


def _build():
    import concourse.bass as bass
    import concourse.tile as tile
    from concourse import bacc, mybir

    f16 = mybir.dt.float16
    f32 = mybir.dt.float32
    AF = mybir.ActivationFunctionType
    OP = mybir.AluOpType

    nc = bacc.Bacc("TRN2", target_bir_lowering=False, debug=False,
                   num_devices=NCORES)

    d_x = nc.dram_tensor("xt", [128, NBLK, HB], f16, kind="ExternalInput").ap()
    d_wr = nc.dram_tensor("wr", [128, 64], f16, kind="ExternalInput").ap()
    d_wz = nc.dram_tensor("wz", [128, 64], f16, kind="ExternalInput").ap()
    d_wn = nc.dram_tensor("wn", [128, 64], f16, kind="ExternalInput").ap()
    d_ohr = nc.dram_tensor("ohr", [128, UNROLL, 64], f16, kind="ExternalInput").ap()
    d_ohz = nc.dram_tensor("ohz", [128, UNROLL, 64], f16, kind="ExternalInput").ap()
    d_ohn = nc.dram_tensor("ohn", [128, UNROLL, 64], f16, kind="ExternalInput").ap()
    d_br = nc.dram_tensor("br", [128, 1], f32, kind="ExternalInput").ap()
    d_bz = nc.dram_tensor("bz", [128, 1], f32, kind="ExternalInput").ap()
    d_bnh = nc.dram_tensor("bnh", [128, 1], f32, kind="ExternalInput").ap()
    d_bni = nc.dram_tensor("bni", [128, 1], f32, kind="ExternalInput").ap()
    d_fcw = nc.dram_tensor("fcw", [128, OUT], f16, kind="ExternalInput").ap()
    d_fcb = nc.dram_tensor("fcb", [128, 2], f32, kind="ExternalInput").ap()
    d_out = nc.dram_tensor("out", [OUT, BC], f32, kind="ExternalOutput").ap()

    with tile.TileContext(nc) as tc, ExitStack() as ctx:
        singles = ctx.enter_context(tc.tile_pool(name="singles", bufs=1))
        work = ctx.enter_context(tc.tile_pool(name="work", bufs=2))
        psum = ctx.enter_context(tc.tile_pool(name="psum", bufs=1, space="PSUM"))

        X = singles.tile([128, NBLK, HB], f16)
        WR = singles.tile([128, 64], f16)
        WZ = singles.tile([128, 64], f16)
        WN = singles.tile([128, 64], f16)
        OHR = singles.tile([128, UNROLL, 64], f16)
        OHZ = singles.tile([128, UNROLL, 64], f16)
        OHN = singles.tile([128, UNROLL, 64], f16)
        BR = singles.tile([128, 1], f32)
        BZ = singles.tile([128, 1], f32)
        BNH = singles.tile([128, 1], f32)
        BNI = singles.tile([128, 1], f32)
        FCW = singles.tile([128, OUT], f16)
        FCB = singles.tile([128, 2], f32)
        H = singles.tile([128, HB], f16)

        for dst, src in ((X, d_x), (WR, d_wr), (WZ, d_wz), (WN, d_wn),
                         (OHR, d_ohr), (OHZ, d_ohz), (OHN, d_ohn),
                         (BR, d_br), (BZ, d_bz), (BNH, d_bnh), (BNI, d_bni),
                         (FCW, d_fcw), (FCB, d_fcb)):
            nc.gpsimd.dma_start(dst[:], src[:])
        nc.vector.memset(H[:], 0.0)

        HG = HB // NGROUP   # free-dim width per pipelined batch group
        m2mul = (nc.gpsimd.tensor_mul if M2ENG == "gpsimd"
                 else nc.vector.tensor_mul)

        def step(q, xsb, g):
            fd = slice(g * HG, (g + 1) * HG)
            bankR = psum.tile([128, HG], f32, tag=f"bankR{g}", name=f"bankR{g}")
            bankZ = psum.tile([128, HG], f32, tag=f"bankZ{g}", name=f"bankZ{g}")
            bankN = psum.tile([128, HG], f32, tag=f"bankN{g}", name=f"bankN{g}")
            bankX = psum.tile([128, HG], f32, tag=f"bankX{g}", name=f"bankX{g}")
            u, v = slice(0, 64), slice(64, 128)
            xu, xv = xsb[u, :, fd], xsb[v, :, fd]
            hu, hv = H[u, fd], H[v, fd]
            utp, vtp = (0, 0), (64, 64)
            # r region first (feeds the sigmoid that heads the chain), then
            # n (feeds stt), then z (feeds the late multiplies).  Within each
            # PSUM region the x one-hot matmul leads: it has no h dependency,
            # so it executes during the previous step's elementwise phase.
            nc.tensor.matmul(bankR[u, :], OHR[u, q, :], xu,
                             start=True, stop=False, tile_position=utp)
            nc.tensor.matmul(bankR[u, :], WR[u, :], hu,
                             start=False, stop=True, tile_position=utp)
            nc.tensor.matmul(bankR[v, :], OHR[v, q, :], xv,
                             start=True, stop=False, tile_position=vtp)
            nc.tensor.matmul(bankR[v, :], WR[v, :], hv,
                             start=False, stop=True, tile_position=vtp)
            nc.tensor.matmul(bankX[u, :], OHN[u, q, :], xu,
                             start=True, stop=True, tile_position=utp)
            nc.tensor.matmul(bankN[u, :], WN[u, :], hu,
                             start=True, stop=True, tile_position=utp)
            nc.tensor.matmul(bankX[v, :], OHN[v, q, :], xv,
                             start=True, stop=True, tile_position=vtp)
            nc.tensor.matmul(bankN[v, :], WN[v, :], hv,
                             start=True, stop=True, tile_position=vtp)
            nc.tensor.matmul(bankZ[u, :], OHZ[u, q, :], xu,
                             start=True, stop=False, tile_position=utp)
            nc.tensor.matmul(bankZ[u, :], WZ[u, :], hu,
                             start=False, stop=True, tile_position=utp)
            nc.tensor.matmul(bankZ[v, :], OHZ[v, q, :], xv,
                             start=True, stop=False, tile_position=vtp)
            nc.tensor.matmul(bankZ[v, :], WZ[v, :], hv,
                             start=False, stop=True, tile_position=vtp)

            SR = work.tile([128, HG], f16, tag=f"SR{g}", name=f"SR{g}")
            SZ = work.tile([128, HG], f16, tag=f"SZ{g}", name=f"SZ{g}")
            SZB = work.tile([128, HG], f16, tag=f"SZB{g}", name=f"SZB{g}")
            T1 = work.tile([128, HG], f16, tag=f"T1{g}", name=f"T1{g}")
            T2 = work.tile([128, HG], f16, tag=f"T2{g}", name=f"T2{g}")
            NN = work.tile([128, HG], f16, tag=f"NN{g}", name=f"NN{g}")
            M1 = work.tile([128, HG], f16, tag=f"M1{g}", name=f"M1{g}")
            M2 = work.tile([128, HG], f16, tag=f"M2{g}", name=f"M2{g}")
            nc.scalar.activation(SR[:], bankR[:], AF.Sigmoid, bias=BR[:])
            nc.scalar.activation(SZ[:], bankZ[:], AF.Sigmoid, bias=BZ[:])
            # T1 = (hn + b_hh_n) * r
            nc.vector.scalar_tensor_tensor(T1[:], bankN[:], BNH[:], SR[:],
                                           op0=OP.add, op1=OP.mult)
            # T2 = T1 + xn
            nc.vector.tensor_add(T2[:], T1[:], bankX[:])
            # n = tanh(T2 + b_ih_n)
            nc.scalar.activation(NN[:], T2[:], AF.Tanh, bias=BNI[:])
            # h' = (1-z)*n + z*h;  z*h runs off the critical chain
            m2mul(M2[:], SZ[:], H[:, fd])
            nc.vector.tensor_scalar(SZB[:], SZ[:], 1.0, -1.0,
                                    op0=OP.subtract, op1=OP.mult)
            nc.vector.tensor_mul(M1[:], SZB[:], NN[:])
            nc.vector.tensor_add(H[:, fd], M1[:], M2[:])

        def body(blk):
            xsb = X[:, blk, :]
            for q in range(UNROLL):
                for g in range(NGROUP):
                    step(q, xsb, g)

        if NBLK == 1:
            body(0)
        else:
            with tc.For_i(0, NBLK, 1,
                          hint_engines=(mybir.EngineType.PE,)) as i:
                body(bass.ds(i, 1))

        # Final FC: out[o, b] = sum_k fc_w[o, k] h[b, k] + fc_b[o]
        for oh in range(2):
            osl = slice(oh * 128, (oh + 1) * 128)
            fc_u = psum.tile([128, HB], f32, tag="bankR0", name="fc_u")
            fc_v = psum.tile([128, HB], f32, tag="bankZ0", name="fc_v")
            nc.tensor.matmul(fc_u[:], FCW[0:64, osl], H[0:64, :],
                             start=True, stop=True, tile_position=(0, 0))
            nc.tensor.matmul(fc_v[:], FCW[64:128, osl], H[64:128, :],
                             start=True, stop=True, tile_position=(64, 0))
            Ou = work.tile([128, HB], f32, tag="Ou", name="Ou")
            Ov = work.tile([128, HB], f32, tag="Ov", name="Ov")
            nc.scalar.activation(Ou[:], fc_u[:], AF.Identity,
                                 bias=FCB[:, oh:oh + 1])
            nc.scalar.activation(Ov[:], fc_v[:], AF.Identity,
                                 bias=FCB[:, oh:oh + 1])
            nc.gpsimd.dma_start(d_out[osl, 0:HB], Ou[:])
            nc.gpsimd.dma_start(d_out[osl, HB:BC], Ov[:])

    nc.compile()
    return nc


def _host_inputs(x, w_ih, w_hh, b_ih, b_hh, fc_w, fc_b):
    """Build the per-core in_maps (numpy, laid out exactly as SBUF tiles)."""
    f16 = np.float16
    f32 = np.float32
    x = np.asarray(x, f32)
    w_ih = np.asarray(w_ih, f32)
    w_hh = np.asarray(w_hh, f32)
    b_ih = np.asarray(b_ih, f32)
    b_hh = np.asarray(b_hh, f32)
    fc_w = np.asarray(fc_w, f32)
    fc_b = np.asarray(fc_b, f32)

    eye = np.eye(UNROLL, dtype=f32)

    def oh(seg):
        w = w_ih[seg, 0]
        o = np.einsum("pq,m->pqm", eye, w)          # [64, UNROLL, 64]
        return np.concatenate([o, o], 0).astype(f16)  # [128, UNROLL, 64]

    def wstack(seg):
        t = w_hh[seg, :].T                            # [64(k), 64(m)]
        return np.vstack([t, t]).astype(f16)

    def btile(v):
        return np.tile(v.reshape(-1, 1), (2, 1)).astype(f32)  # [128, 1]

    shared = {
        "wr": wstack(slice(0, 64)),
        "wz": wstack(slice(64, 128)),
        "wn": wstack(slice(128, 192)),
        "ohr": oh(slice(0, 64)),
        "ohz": oh(slice(64, 128)),
        "ohn": oh(slice(128, 192)),
        "br": btile(b_ih[0:64] + b_hh[0:64]),
        "bz": btile(b_ih[64:128] + b_hh[64:128]),
        "bnh": btile(b_hh[128:192]),
        "bni": btile(b_ih[128:192]),
        "fcw": np.vstack([fc_w.T, fc_w.T]).astype(f16),  # [128, 256]
        "fcb": np.stack([fc_b[0:128], fc_b[128:256]], 1).astype(f32),
    }

    in_maps = []
    for c in range(NCORES):
        xs = x[c * BC:(c + 1) * BC, :T, 0]            # [BC b, T t]
        xT = np.ascontiguousarray(xs.T)               # [T, BC]
        xr = xT.reshape(NBLK, UNROLL, BC)             # [blk, p, b]
        lo = xr[:, :, 0:HB].transpose(1, 0, 2)        # [64, blk, HB]
        hi = xr[:, :, HB:BC].transpose(1, 0, 2)
        Xh = np.ascontiguousarray(
            np.concatenate([lo, hi], 0)).astype(f16)  # [128, blk, HB]
        m = dict(shared)
        m["xt"] = Xh
        in_maps.append(m)
    return in_maps


def _run(in_maps, trace=False):
    from concourse import bass_utils
    if "nc" not in _CACHE:
        _CACHE["nc"] = _build()
    nc = _CACHE["nc"]
    res = bass_utils.run_bass_kernel_spmd(
        nc, in_maps, core_ids=list(range(NCORES)), trace=trace)
    return res


def kernel(**inputs):
    in_maps = _host_inputs(**inputs)
    res = _run(in_maps, trace=False)
    out = np.empty([B, OUT], np.float32)
    for c in range(NCORES):
        out[c * BC:(c + 1) * BC, :] = res.results[c]["out"].T
    return out
